# revision 16
# baseline (speedup 1.0000x reference)
"""Trainium2 Bass kernel for nn_Conv_M_49409303773352.

Strategy (data-parallel over batch x H-halves -> 8 shards):
  Per position p=(b,h,w): feat = [x-patches(576), m-patches(576)] (3x3, edge pad)
  w1 = feat@W1+b1 [576]; w2 = feat@W2+b2 [4096]
  yr_c = sum_k yp*w1 ; mr_c = sum_k |mp*w1| ; sr_c = sum_k |sp*w1|   (mp,sp>=0)
  y_o  = sum_c yr_c*w2[c,o] ; m_y = (sum_c mr_c|w2|)/(sum_c sr_c|w2|)

Device schedule (per 128-position row tile), v2 -- PE is the sole
bottleneck (~21.5us/row); everything else hides under it:
  PE   : 9-tap channel contraction (128 partitions = 64 x + 64 m chans)
         into PSUM, 1024-col chunks, 2 phases {w1,q0,q1} / {q2,q3};
         bias rows via ones-outer-product matmuls FIRST in each
         accumulation group (start=True), taps accumulate on top.
  ACT  : PSUM->SBUF bf16 downconverts: signed w2b (y path) AND |w2b|
         (shared by m,s paths via func=Abs).
  Pool : t1 mults (pat * w1b) into a path-major [128,3,576] tile.
  DVE  : t1 reduces (abs for m/s), three 2x-mode stride-1 t2 mults
         (in-place on materialized replicas), batched bf16 tree-adds
         (all 3 paths in one instruction) + final fp32 tensor_reduce,
         reciprocal + m_y multiply.
  DMA  : broadcast-materializes yr/mr/sr into the [128,3,64,64] T tile
         (replica per output channel) so the t2 mults avoid the 1x-mode
         broadcast-AP penalty on DVE.
All wire data bf16, fp32 accumulation in PSUM/final reduce.

build_program(reps=N) unrolls the whole row loop N times inside one NEFF
(outputs are recomputed identically each rep) so steady-state per-execution
time can be measured from dispatch-count slopes without per-dispatch RPC
overhead dominating.
"""
import sys
sys.path.insert(0, '/opt/trn_rl_repo')
import os
import numpy as np
import ml_dtypes

import concourse.bass as bass
import concourse.mybir as mybir
import concourse.tile as tile
from concourse.tile import TileContext
from concourse.vector_clock import ScopedClock
from concourse.bass_utils import run_bass_kernel_spmd

BF = ml_dtypes.bfloat16
BF_DT = mybir.dt.bfloat16
F32 = mybir.dt.float32

B, C, H, W = 4, 64, 128, 128
ROWS = int(os.environ.get("KERNEL_ROWS", "64"))   # output rows per core
N_CORES = 8
K2 = 9
F1 = 576          # K2*C
F2 = 4096         # C*C


# ---- walrus only accepts ONE sem wait per instruction: split the final drain
def _split_drain_and_barrier(self, tick_clock, wait_clock):
    nc = self.nc
    probe = nc.sync.nop()
    wait_clock.add_sem_waits(probe.ins, ScopedClock({None: tick_clock.global_clock}))
    waits = list(probe.ins.sync_info.on_wait)
    if len(waits) > 1:
        probe.ins.sync_info.on_wait = waits[:1]
        for w in waits[1:]:
            extra = nc.sync.nop()
            extra.ins.sync_info = probe.ins.sync_info.__class__(
                on_wait=[w], on_update=[])
    nc.sync.drain()
    nc.all_engine_barrier()
    assert self.sems is not None
    popped = nc._tile_sem_poison_stack.pop()
    assert popped is self._sem_poison
    nc.clear_and_free_semaphores(list(self.sems.allocated().values()))
    nc.all_engine_barrier()


tile.TileContext._drain_and_barrier = _split_drain_and_barrier


def _split_multi_sync(nc):
    """Walrus accepts one sync wait (and update) per instruction: hoist extras
    onto same-engine nops inserted just before (waits) / after (updates)."""
    def make_nop(engine, si_cls, waits=(), updates=()):
        bi = nc.engines[engine].nop()
        blk = nc.cur_bb.bb
        assert blk.instructions[-1] is bi.ins
        blk.instructions.pop()
        bi.ins.sync_info = si_cls(on_wait=list(waits), on_update=list(updates))
        return bi.ins

    for blk in nc.m.functions[0].blocks:
        out = []
        for inst in blk.instructions:
            si = getattr(inst, "sync_info", None)
            if si is None:
                out.append(inst)
                continue
            waits = list(si.on_wait or [])
            updates = list(si.on_update or [])
            extra_w = waits[:-1] if len(waits) > 1 else []
            extra_u = updates[1:] if len(updates) > 1 else []
            if extra_w:
                for w in extra_w:
                    out.append(make_nop(inst.engine, si.__class__, waits=[w]))
                si.on_wait = waits[-1:]
            out.append(inst)
            if extra_u:
                assert inst.opcode not in ("DMACopy", "DMATranspose"), \
                    "cannot defer DMA completion updates"
                si.on_update = updates[:1]
                for u in extra_u:
                    out.append(make_nop(inst.engine, si.__class__, updates=[u]))
        blk.instructions[:] = out


def _dedup_ldweights(nc):
    """Drop Ldweights whose stationary AP equals the previous Ldweights in
    the stream (PE array retains weights across the paired Matmults).
    Conservative: keep any Ldweights carrying sync waits/updates."""
    n_drop = 0
    for blk in nc.m.functions[0].blocks:
        out = []
        last_key = None
        for inst in blk.instructions:
            if inst.opcode == "Ldweights":
                si = getattr(inst, "sync_info", None)
                has_sync = si is not None and (si.on_wait or si.on_update)
                key = str(inst.ins[0])
                if key == last_key and not has_sync:
                    n_drop += 1
                    continue
                last_key = key
            out.append(inst)
        blk.instructions[:] = out
    return n_drop


def build_program(reps=1, skip=()):
    nc = bass.Bass()
    xm_d = nc.dram_tensor("xm", [2 * C, ROWS + 2, W + 2], BF_DT, kind="ExternalInput")
    ypat_d = nc.dram_tensor("ypat", [ROWS, W, F1], BF_DT, kind="ExternalInput")
    mpat_d = nc.dram_tensor("mpat", [ROWS, W, F1], BF_DT, kind="ExternalInput")
    spat_d = nc.dram_tensor("spat", [ROWS, W, F1], BF_DT, kind="ExternalInput")
    w1k_d = nc.dram_tensor("w1k", [K2, 128, F1], BF_DT, kind="ExternalInput")
    w2k_d = nc.dram_tensor("w2k", [K2, 128, F2], BF_DT, kind="ExternalInput")
    b1_d = nc.dram_tensor("b1", [1, F1], BF_DT, kind="ExternalInput")
    b2_d = nc.dram_tensor("b2", [1, F2], BF_DT, kind="ExternalInput")
    y_d = nc.dram_tensor("y", [ROWS, W, C], F32, kind="ExternalOutput")
    my_d = nc.dram_tensor("my", [ROWS, W, C], F32, kind="ExternalOutput")

    rep_mode = os.environ.get("KERNEL_REP", "mix")  # mix|dma|gpsimd|act|dve

    with TileContext(nc) as tc:
        with (
            tc.tile_pool(name="wts", bufs=1) as wts,
            tc.tile_pool(name="rows", bufs=4) as rows,
            tc.tile_pool(name="pats", bufs=2) as pats,
            tc.tile_pool(name="mid", bufs=2) as mid,
            tc.tile_pool(name="sml", bufs=3) as sml,
            tc.tile_pool(name="ps1p", bufs=1, space="PSUM") as ps1p,
            tc.tile_pool(name="qp", bufs=3, space="PSUM") as qp,
        ):
            w1k = wts.tile([128, K2, F1], BF_DT)
            nc.sync.dma_start(out=w1k, in_=w1k_d[:, :, :].rearrange("k p f -> p k f"))
            w2k = wts.tile([128, K2, F2], BF_DT)
            nc.sync.dma_start(out=w2k, in_=w2k_d[:, :, :].rearrange("k p f -> p k f"))
            b1s = wts.tile([1, F1], BF_DT)
            nc.sync.dma_start(out=b1s, in_=b1_d[:, :])
            b2s = wts.tile([1, F2], BF_DT)
            nc.sync.dma_start(out=b2s, in_=b2_d[:, :])
            ones = wts.tile([1, 128], BF_DT)
            nc.vector.memset(ones, 1.0)

            for rep in range(reps):
                for h in range(ROWS):
                    xmr = rows.tile([128, 3, W + 2], BF_DT)
                    nc.sync.dma_start(out=xmr, in_=xm_d[:, h:h + 3, :])
                    ypt = pats.tile([128, F1], BF_DT)
                    nc.sync.dma_start(out=ypt, in_=ypat_d[h, :, :])
                    mpt = pats.tile([128, F1], BF_DT)
                    nc.sync.dma_start(out=mpt, in_=mpat_d[h, :, :])
                    spt = pats.tile([128, F1], BF_DT)
                    nc.sync.dma_start(out=spt, in_=spat_d[h, :, :])

                    taps = tuple(range(K2))
                    w2b = mid.tile([128, F2], BF_DT, tag="w2b", bufs=3)
                    aw2 = mid.tile([128, F2], BF_DT, tag="aw2", bufs=2)

                    # ---- phase A: accumulation groups {w1(576), q0, q1};
                    # bias rows first (start=True) via ones outer-product,
                    # then the 9 taps accumulate on top.
                    ps1 = ps1p.tile([128, 1024], F32, tag="ps1")
                    qA = [qp.tile([128, 1024], F32, tag="q", name=f"q{qq}")
                          for qq in range(2)]
                    if "gemm" not in skip:
                        for lo, hi in ((0, 512), (512, F1)):
                            nc.tensor.matmul(ps1[:, lo:hi], ones[0:1, :],
                                             b1s[0:1, lo:hi], start=True,
                                             stop=False)
                        for qq in range(2):
                            for j2 in range(2):
                                lo = qq * 1024 + j2 * 512
                                nc.tensor.matmul(
                                    qA[qq][:, j2 * 512:(j2 + 1) * 512],
                                    ones[0:1, :], b2s[0:1, lo:lo + 512],
                                    start=True, stop=False)
                        for k in taps:
                            kh, kw = divmod(k, 3)
                            st = xmr[:, kh, kw:kw + 128]
                            last = k == taps[-1]
                            for lo, hi in ((0, 512), (512, F1)):
                                nc.tensor.matmul(ps1[:, lo:hi], st,
                                                 w1k[:, k, lo:hi],
                                                 start=False, stop=last)
                            for qq in range(2):
                                for j2 in range(2):
                                    lo = qq * 1024 + j2 * 512
                                    nc.tensor.matmul(
                                        qA[qq][:, j2 * 512:(j2 + 1) * 512],
                                        st, w2k[:, k, lo:lo + 512],
                                        start=False, stop=last)
                    signed_eng = os.environ.get("KERNEL_SIGNED", "act")
                    if "act" not in skip:
                        w1b = mid.tile([128, F1], BF_DT, tag="w1b", bufs=2)
                        nc.scalar.copy(out=w1b, in_=ps1[:, 0:F1])
                        for qq in range(2):
                            sl = slice(qq * 1024, (qq + 1) * 1024)
                            if signed_eng == "gpsimd":
                                nc.gpsimd.tensor_copy(w2b[:, sl], qA[qq])
                            else:
                                nc.scalar.copy(out=w2b[:, sl], in_=qA[qq])
                            nc.scalar.activation(
                                out=aw2[:, sl], in_=qA[qq],
                                func=mybir.ActivationFunctionType.Abs)

                    # ---- phase B: groups {q2, q3}, bias first
                    qB = [qp.tile([128, 1024], F32, tag="q", name=f"q{qq+2}")
                          for qq in range(2)]
                    if "gemm" not in skip:
                        for qq in range(2):
                            for j2 in range(2):
                                lo = 2048 + qq * 1024 + j2 * 512
                                nc.tensor.matmul(
                                    qB[qq][:, j2 * 512:(j2 + 1) * 512],
                                    ones[0:1, :], b2s[0:1, lo:lo + 512],
                                    start=True, stop=False)
                        for k in taps:
                            kh, kw = divmod(k, 3)
                            st = xmr[:, kh, kw:kw + 128]
                            last = k == taps[-1]
                            for qq in range(2):
                                for j2 in range(2):
                                    lo = 2048 + qq * 1024 + j2 * 512
                                    nc.tensor.matmul(
                                        qB[qq][:, j2 * 512:(j2 + 1) * 512],
                                        st, w2k[:, k, lo:lo + 512],
                                        start=False, stop=last)
                    if "act" not in skip:
                        for qq in range(2):
                            sl = slice(2048 + qq * 1024, 2048 + (qq + 1) * 1024)
                            if signed_eng == "gpsimd":
                                nc.gpsimd.tensor_copy(w2b[:, sl], qB[qq])
                            else:
                                nc.scalar.copy(out=w2b[:, sl], in_=qB[qq])
                            nc.scalar.activation(
                                out=aw2[:, sl], in_=qB[qq],
                                func=mybir.ActivationFunctionType.Abs)

                    if "apply" in skip or "act" in skip:
                        continue

                    with nc.allow_low_precision("bf16 intermediates validated "
                                                "against 2e-2 tolerance"):
                        # ---- t1: pat * w1b on Pool, path-major tile
                        t1 = mid.tile([128, 3, F1], BF_DT, tag="t1", bufs=2)
                        nc.gpsimd.tensor_mul(t1[:, 0, :], ypt, w1b)
                        nc.gpsimd.tensor_mul(t1[:, 1, :], mpt, w1b)
                        nc.gpsimd.tensor_mul(t1[:, 2, :], spt, w1b)
                        # ---- tap reduces on DVE (abs for m/s)
                        red = sml.tile([128, 3, C], BF_DT, tag="red")
                        nc.vector.tensor_reduce(
                            out=red[:, 0:1, :],
                            in_=t1[:, 0:1, :].rearrange(
                                "p a (c k) -> p a c k", k=K2),
                            axis=mybir.AxisListType.X, op=mybir.AluOpType.add)
                        nc.vector.tensor_reduce(
                            out=red[:, 1:3, :],
                            in_=t1[:, 1:3, :].rearrange(
                                "p a (c k) -> p a c k", k=K2),
                            axis=mybir.AxisListType.X, op=mybir.AluOpType.add,
                            apply_absolute_value=True)

                        # ---- materialize yr/mr/sr replicas into T
                        # (engine copies: a broadcast DMA shatters into 128B
                        # descriptors -- 1.6M per exec -- and its latency
                        # backpressures PE through the mid-pool recycle)
                        T = mid.tile([128, 3, C, C], BF_DT, tag="T")
                        rep_engs = {"mix": ("gpsimd", "act", "act"),
                                    "gpsimd": ("gpsimd",) * 3,
                                    "act": ("act",) * 3,
                                    "dma": ("dma",) * 3,
                                    "dve": ("dve",) * 3}[rep_mode]
                        for ci in range(3):
                            src = red[:, ci:ci + 1, :].to_broadcast([128, C, C])
                            dst = T[:, ci, :, :]
                            if rep_engs[ci] == "dma":
                                nc.sync.dma_start(out=dst, in_=src)
                            elif rep_engs[ci] == "gpsimd":
                                nc.gpsimd.tensor_copy(dst, src)
                            elif rep_engs[ci] == "act":
                                nc.scalar.copy(out=dst, in_=src)
                            # "dve": no materialize; mults below use bcast

                        # ---- t2 mults (2x-mode stride-1, in place on T)
                        w2v = w2b[:, :].rearrange("p (o c) -> p o c", c=C)
                        av2 = aw2[:, :].rearrange("p (o c) -> p o c", c=C)
                        if rep_mode == "dve":
                            for ci, wv in ((0, w2v), (1, av2), (2, av2)):
                                nc.vector.tensor_mul(
                                    T[:, ci, :, :], wv,
                                    red[:, ci:ci + 1, :].to_broadcast(
                                        [128, C, C]))
                        else:
                            for ci, wv in ((0, w2v), (1, av2), (2, av2)):
                                nc.vector.tensor_mul(
                                    T[:, ci, :, :], T[:, ci, :, :], wv)

                        # ---- batched tree-adds (all 3 paths per inst), then
                        # one fp32 tensor_reduce over the last 8
                        w = C
                        for _ in range(3):
                            w //= 2
                            nc.vector.tensor_add(
                                T[:, :, :, 0:w], T[:, :, :, 0:w],
                                T[:, :, :, w:2 * w])
                        acc3 = sml.tile([128, 3, C], F32, tag="acc3")
                        nc.vector.tensor_reduce(
                            out=acc3, in_=T[:, :, :, 0:w],
                            axis=mybir.AxisListType.X, op=mybir.AluOpType.add)

                        srec = sml.tile([128, C], F32, tag="srec")
                        nc.vector.reciprocal(out=srec, in_=acc3[:, 2, :])
                        my_t = sml.tile([128, C], F32, tag="my")
                        nc.vector.tensor_mul(my_t, acc3[:, 1, :], srec)
                        nc.sync.dma_start(out=y_d[h, :, :], in_=acc3[:, 0, :])
                        nc.sync.dma_start(out=my_d[h, :, :], in_=my_t)
    _split_multi_sync(nc)
    _dedup_ldweights(nc)
    return nc


def _row_gather(Wm, k):
    # rows of W (1152) feeding tap k for channels [x 0..63, m 0..63]
    idx = np.concatenate([np.arange(64) * 9 + k, 576 + np.arange(64) * 9 + k])
    return Wm[idx]


def _unfold(t):
    # t [B,C,H,W] fp32 -> patches [B,H,W,C*9] bf16 (index c*9 + kh*3 + kw)
    tp = np.pad(t, ((0, 0), (0, 0), (1, 1), (1, 1)), mode='edge').astype(BF)
    win = np.lib.stride_tricks.sliding_window_view(tp, (3, 3), axis=(2, 3))
    # win: [B,C,H,W,3,3] -> [B,H,W,C,3,3]
    return np.ascontiguousarray(win.transpose(0, 2, 3, 1, 4, 5)).reshape(B, H, W, C * 9)


def prepare_in_maps(x, m, s, W1, b1, W2, b2):
    x = np.asarray(x, np.float32); m = np.asarray(m, np.float32)
    s = np.asarray(s, np.float32)
    W1 = np.asarray(W1, np.float32); W2 = np.asarray(W2, np.float32)
    b1 = np.asarray(b1, np.float32); b2 = np.asarray(b2, np.float32)

    # W2 cols permuted from [c,o] to [o,c]; biases likewise
    W2p = W2.reshape(1152, C, C).transpose(0, 2, 1).reshape(1152, F2)
    b2p = b2.reshape(C, C).T.reshape(1, F2).astype(BF)
    w1k = np.stack([_row_gather(W1.astype(BF), k) for k in range(K2)])
    w2k = np.stack([_row_gather(W2p.astype(BF), k) for k in range(K2)])
    b1h = b1.reshape(1, F1).astype(BF)

    xmp = np.pad(np.concatenate([x, m], axis=1),
                 ((0, 0), (0, 0), (1, 1), (1, 1)), mode='edge').astype(BF)
    ypat = _unfold(x); mpat = _unfold(m); spat = _unfold(s)

    in_maps = []
    shards = []
    for core in range(N_CORES):
        b, half = divmod(core, 2)
        h0 = half * (H // 2)
        shards.append((b, h0))
        in_maps.append({
            "xm": np.ascontiguousarray(xmp[b, :, h0:h0 + ROWS + 2, :]),
            "ypat": np.ascontiguousarray(ypat[b, h0:h0 + ROWS].reshape(ROWS, W, F1)),
            "mpat": np.ascontiguousarray(mpat[b, h0:h0 + ROWS].reshape(ROWS, W, F1)),
            "spat": np.ascontiguousarray(spat[b, h0:h0 + ROWS].reshape(ROWS, W, F1)),
            "w1k": w1k, "w2k": w2k, "b1": b1h, "b2": b2p,
        })
    return in_maps, shards


_DEV_IN_CACHE = {}


def _time_hw(in_maps, reps, k1, k2, skip=()):
    """Steady-state per-execution time: build a reps-unrolled NEFF, stage
    inputs on device once, then measure the wall-clock slope between k1 and
    k2 pipelined dispatches. Returns ns per single kernel execution."""
    import time
    import jax
    from jax.sharding import Mesh, PartitionSpec, NamedSharding
    from jax.experimental.shard_map import shard_map
    from concourse.bass2jax import (_bass_exec_p, partition_id_tensor,
                                    install_neuronx_cc_hook)

    nc = build_program(reps=reps, skip=skip)
    install_neuronx_cc_hook()
    partition_name = (nc.partition_id_tensor.name
                      if nc.partition_id_tensor else None)
    in_names, out_names, out_avals, zero_shapes, zero_dtypes = [], [], [], [], []
    for alloc in nc.m.functions[0].allocations:
        if not isinstance(alloc, mybir.MemoryLocationSet):
            continue
        name = alloc.memorylocations[0].name
        if alloc.kind == "ExternalInput":
            if name != partition_name:
                in_names.append(name)
        elif alloc.kind == "ExternalOutput":
            out_names.append(name)
            shape = tuple(alloc.tensor_shape)
            dtype = mybir.dt.np(alloc.dtype)
            out_avals.append(jax.core.ShapedArray(shape, dtype))
            zero_shapes.append((N_CORES * shape[0], *shape[1:]))
            zero_dtypes.append(dtype)
    n_params = len(in_names)
    all_in_names = list(in_names) + list(out_names)
    if partition_name is not None:
        all_in_names.append(partition_name)

    def _body(*args):
        operands = list(args)
        if partition_name is not None:
            operands.append(partition_id_tensor())
        return tuple(_bass_exec_p.bind(
            *operands, out_avals=tuple(out_avals), in_names=tuple(all_in_names),
            out_names=tuple(out_names), lowering_input_output_aliases=(),
            sim_require_finite=True, sim_require_nnan=True, nc=nc))

    import functools
    import jax.numpy as jnp
    devices = jax.devices()[:N_CORES]
    mesh = Mesh(np.asarray(devices), ("core",))
    # no donation in the timing path: without declared IO aliasing the
    # custom-call results are fresh (uninit) buffers the NEFF fully writes,
    # so one static zero-set can serve every dispatch.
    fn = jax.jit(
        shard_map(_body, mesh=mesh,
                  in_specs=(PartitionSpec("core"),) * (n_params + len(out_names)),
                  out_specs=(PartitionSpec("core"),) * len(out_names),
                  check_rep=False),
        keep_unused=True)
    shard = NamedSharding(mesh, PartitionSpec("core"))
    global _DEV_IN_CACHE
    key = tuple(in_names)
    if _DEV_IN_CACHE.get("key") != key:
        _DEV_IN_CACHE = {"key": key, "arrs": [jax.device_put(
            np.concatenate([np.asarray(in_maps[c][nm]) for c in range(N_CORES)],
                           axis=0), shard) for nm in in_names]}
        jax.block_until_ready(_DEV_IN_CACHE["arrs"])
    dev_in = _DEV_IN_CACHE["arrs"]

    # allocate output buffers directly on device (no 32MB H2D per set)
    zfns = [jax.jit(functools.partial(jnp.zeros, s, d), out_shardings=shard)
            for s, d in zip(zero_shapes, zero_dtypes)]

    def fresh_zeros():
        zs = [f() for f in zfns]
        jax.block_until_ready(zs)
        return zs

    # warmup: trace + NEFF compile + first exec
    zs = fresh_zeros()
    outs = fn(*dev_in, *zs)
    jax.block_until_ready(outs)

    def timed(K):
        t0 = time.perf_counter()
        outs_l = [fn(*dev_in, *zs) for _ in range(K)]
        jax.block_until_ready(outs_l)
        return time.perf_counter() - t0

    slopes = []
    for _ in range(3):
        ta, tb = timed(k1), timed(k2)
        slopes.append((tb - ta) / ((k2 - k1) * reps))
    slopes.sort()
    per_exec = slopes[1]
    print(f"[time_hw] reps={reps} slopes(us)="
          f"{[int(s * 1e6) for s in slopes]} -> per-exec {per_exec*1e6:.0f}us")
    return int(per_exec * 1e9)


def kernel(x, m, s, W1, b1, W2, b2):
    in_maps, shards = prepare_in_maps(x, m, s, W1, b1, W2, b2)
    nc = build_program()
    res = run_bass_kernel_spmd(nc, in_maps, core_ids=list(range(N_CORES)),
                               trace=False)
    if os.environ.get("KERNEL_TIME"):
        reps = int(os.environ.get("KERNEL_TIME_REPS", "16"))
        ns = _time_hw(in_maps, reps=reps, k1=2, k2=8)
        with open("/tmp/kernel_exec_time.txt", "w") as f:
            f.write(str(ns))

    y = np.zeros((B, C, H, W), np.float32)
    m_y = np.zeros((B, C, H, W), np.float32)
    for core, (b, h0) in enumerate(shards):
        out = res.results[core]
        y[b, :, h0:h0 + ROWS, :] = np.asarray(
            out["y"], np.float32).transpose(2, 0, 1)
        m_y[b, :, h0:h0 + ROWS, :] = out["my"].transpose(2, 0, 1)
    return y, m_y, np.ones_like(m_y)


# revision 17
# speedup vs baseline: 1.1632x; 1.1632x over previous
"""Trainium2 Bass kernel for nn_Conv_M_49409303773352.

Strategy (data-parallel over batch x H-halves -> 8 shards):
  Per position p=(b,h,w): feat = [x-patches(576), m-patches(576)] (3x3, edge pad)
  w1 = feat@W1+b1 [576]; w2 = feat@W2+b2 [4096]
  yr_c = sum_k yp*w1 ; mr_c = sum_k |mp*w1| ; sr_c = sum_k |sp*w1|   (mp,sp>=0)
  y_o  = sum_c yr_c*w2[c,o] ; m_y = (sum_c mr_c|w2|)/(sum_c sr_c|w2|)

Device schedule (per 128-position row tile), v2 -- PE is the sole
bottleneck (~21.5us/row); everything else hides under it:
  PE   : 9-tap channel contraction (128 partitions = 64 x + 64 m chans)
         into PSUM, 1024-col chunks, 2 phases {w1,q0,q1} / {q2,q3};
         bias rows via ones-outer-product matmuls FIRST in each
         accumulation group (start=True), taps accumulate on top.
  ACT  : PSUM->SBUF bf16 downconverts: signed w2b (y path) AND |w2b|
         (shared by m,s paths via func=Abs).
  Pool : t1 mults (pat * w1b) into a path-major [128,3,576] tile.
  DVE  : t1 reduces (abs for m/s), three 2x-mode stride-1 t2 mults
         (in-place on materialized replicas), batched bf16 tree-adds
         (all 3 paths in one instruction) + final fp32 tensor_reduce,
         reciprocal + m_y multiply.
  DMA  : broadcast-materializes yr/mr/sr into the [128,3,64,64] T tile
         (replica per output channel) so the t2 mults avoid the 1x-mode
         broadcast-AP penalty on DVE.
All wire data bf16, fp32 accumulation in PSUM/final reduce.

build_program(reps=N) unrolls the whole row loop N times inside one NEFF
(outputs are recomputed identically each rep) so steady-state per-execution
time can be measured from dispatch-count slopes without per-dispatch RPC
overhead dominating.
"""
import sys
sys.path.insert(0, '/opt/trn_rl_repo')
import os
import numpy as np
import ml_dtypes

import concourse.bass as bass
import concourse.mybir as mybir
import concourse.tile as tile
from concourse.tile import TileContext
from concourse.vector_clock import ScopedClock
from concourse.bass_utils import run_bass_kernel_spmd

BF = ml_dtypes.bfloat16
BF_DT = mybir.dt.bfloat16
F32 = mybir.dt.float32

B, C, H, W = 4, 64, 128, 128
ROWS = int(os.environ.get("KERNEL_ROWS", "64"))   # output rows per core
N_CORES = 8
K2 = 9
F1 = 576          # K2*C
F2 = 4096         # C*C


# ---- walrus only accepts ONE sem wait per instruction: split the final drain
def _split_drain_and_barrier(self, tick_clock, wait_clock):
    nc = self.nc
    probe = nc.sync.nop()
    wait_clock.add_sem_waits(probe.ins, ScopedClock({None: tick_clock.global_clock}))
    waits = list(probe.ins.sync_info.on_wait)
    if len(waits) > 1:
        probe.ins.sync_info.on_wait = waits[:1]
        for w in waits[1:]:
            extra = nc.sync.nop()
            extra.ins.sync_info = probe.ins.sync_info.__class__(
                on_wait=[w], on_update=[])
    nc.sync.drain()
    nc.all_engine_barrier()
    assert self.sems is not None
    popped = nc._tile_sem_poison_stack.pop()
    assert popped is self._sem_poison
    nc.clear_and_free_semaphores(list(self.sems.allocated().values()))
    nc.all_engine_barrier()


tile.TileContext._drain_and_barrier = _split_drain_and_barrier


def _split_multi_sync(nc):
    """Walrus accepts one sync wait (and update) per instruction: hoist extras
    onto same-engine nops inserted just before (waits) / after (updates)."""
    def make_nop(engine, si_cls, waits=(), updates=()):
        bi = nc.engines[engine].nop()
        blk = nc.cur_bb.bb
        assert blk.instructions[-1] is bi.ins
        blk.instructions.pop()
        bi.ins.sync_info = si_cls(on_wait=list(waits), on_update=list(updates))
        return bi.ins

    for blk in nc.m.functions[0].blocks:
        out = []
        for inst in blk.instructions:
            si = getattr(inst, "sync_info", None)
            if si is None:
                out.append(inst)
                continue
            waits = list(si.on_wait or [])
            updates = list(si.on_update or [])
            extra_w = waits[:-1] if len(waits) > 1 else []
            extra_u = updates[1:] if len(updates) > 1 else []
            if extra_w:
                for w in extra_w:
                    out.append(make_nop(inst.engine, si.__class__, waits=[w]))
                si.on_wait = waits[-1:]
            out.append(inst)
            if extra_u:
                assert inst.opcode not in ("DMACopy", "DMATranspose"), \
                    "cannot defer DMA completion updates"
                si.on_update = updates[:1]
                for u in extra_u:
                    out.append(make_nop(inst.engine, si.__class__, updates=[u]))
        blk.instructions[:] = out


def _dedup_ldweights(nc):
    """Drop Ldweights whose stationary AP equals the previous Ldweights in
    the stream (PE array retains weights across the paired Matmults).
    Conservative: keep any Ldweights carrying sync waits/updates."""
    n_drop = 0
    for blk in nc.m.functions[0].blocks:
        out = []
        last_key = None
        for inst in blk.instructions:
            if inst.opcode == "Ldweights":
                si = getattr(inst, "sync_info", None)
                has_sync = si is not None and (si.on_wait or si.on_update)
                key = str(inst.ins[0])
                if key == last_key and not has_sync:
                    n_drop += 1
                    continue
                last_key = key
            out.append(inst)
        blk.instructions[:] = out
    return n_drop


def build_program(reps=1, skip=()):
    nc = bass.Bass()
    xm_d = nc.dram_tensor("xm", [2 * C, ROWS + 2, W + 2], BF_DT, kind="ExternalInput")
    ypat_d = nc.dram_tensor("ypat", [ROWS, W, F1], BF_DT, kind="ExternalInput")
    mpat_d = nc.dram_tensor("mpat", [ROWS, W, F1], BF_DT, kind="ExternalInput")
    spat_d = nc.dram_tensor("spat", [ROWS, W, F1], BF_DT, kind="ExternalInput")
    w1k_d = nc.dram_tensor("w1k", [K2, 128, F1], BF_DT, kind="ExternalInput")
    w2k_d = nc.dram_tensor("w2k", [K2, 128, F2], BF_DT, kind="ExternalInput")
    b1_d = nc.dram_tensor("b1", [1, F1], BF_DT, kind="ExternalInput")
    b2_d = nc.dram_tensor("b2", [1, F2], BF_DT, kind="ExternalInput")
    y_d = nc.dram_tensor("y", [ROWS, W, C], F32, kind="ExternalOutput")
    my_d = nc.dram_tensor("my", [ROWS, W, C], F32, kind="ExternalOutput")

    rep_mode = os.environ.get("KERNEL_REP", "dma")  # mix|dma|gpsimd|act|dve

    with TileContext(nc) as tc:
        with (
            tc.tile_pool(name="wts", bufs=1) as wts,
            tc.tile_pool(name="rows", bufs=4) as rows,
            tc.tile_pool(name="pats", bufs=3) as pats,
            tc.tile_pool(name="mid", bufs=2) as mid,
            tc.tile_pool(name="sml", bufs=3) as sml,
            tc.tile_pool(name="ps1p", bufs=1, space="PSUM") as ps1p,
            tc.tile_pool(name="qp", bufs=3, space="PSUM") as qp,
        ):
            w1k = wts.tile([128, K2, F1], BF_DT)
            nc.sync.dma_start(out=w1k, in_=w1k_d[:, :, :].rearrange("k p f -> p k f"))
            w2k = wts.tile([128, K2, F2], BF_DT)
            nc.sync.dma_start(out=w2k, in_=w2k_d[:, :, :].rearrange("k p f -> p k f"))
            b1s = wts.tile([1, F1], BF_DT)
            nc.sync.dma_start(out=b1s, in_=b1_d[:, :])
            b2s = wts.tile([1, F2], BF_DT)
            nc.sync.dma_start(out=b2s, in_=b2_d[:, :])
            ones = wts.tile([1, 128], BF_DT)
            nc.vector.memset(ones, 1.0)

            for rep in range(reps):
                for h in range(ROWS):
                    xmr = rows.tile([128, 3, W + 2], BF_DT)
                    nc.sync.dma_start(out=xmr, in_=xm_d[:, h:h + 3, :])
                    ypt = pats.tile([128, F1], BF_DT)
                    nc.sync.dma_start(out=ypt, in_=ypat_d[h, :, :])
                    mpt = pats.tile([128, F1], BF_DT)
                    nc.sync.dma_start(out=mpt, in_=mpat_d[h, :, :])
                    spt = pats.tile([128, F1], BF_DT)
                    nc.sync.dma_start(out=spt, in_=spat_d[h, :, :])

                    taps = tuple(range(K2))
                    w2b = mid.tile([128, F2], BF_DT, tag="w2b", bufs=2)
                    aw2 = mid.tile([128, F2], BF_DT, tag="aw2", bufs=2)

                    # ---- phase A: accumulation groups {w1(576), q0, q1};
                    # bias rows first (start=True) via ones outer-product,
                    # then the 9 taps accumulate on top.
                    ps1 = ps1p.tile([128, 1024], F32, tag="ps1")
                    qA = [qp.tile([128, 1024], F32, tag="q", name=f"q{qq}")
                          for qq in range(2)]
                    if "gemm" not in skip:
                        for lo, hi in ((0, 512), (512, F1)):
                            nc.tensor.matmul(ps1[:, lo:hi], ones[0:1, :],
                                             b1s[0:1, lo:hi], start=True,
                                             stop=False)
                        for qq in range(2):
                            for j2 in range(2):
                                lo = qq * 1024 + j2 * 512
                                nc.tensor.matmul(
                                    qA[qq][:, j2 * 512:(j2 + 1) * 512],
                                    ones[0:1, :], b2s[0:1, lo:lo + 512],
                                    start=True, stop=False)
                        for k in taps:
                            kh, kw = divmod(k, 3)
                            st = xmr[:, kh, kw:kw + 128]
                            last = k == taps[-1]
                            for lo, hi in ((0, 512), (512, F1)):
                                nc.tensor.matmul(ps1[:, lo:hi], st,
                                                 w1k[:, k, lo:hi],
                                                 start=False, stop=last)
                            for qq in range(2):
                                for j2 in range(2):
                                    lo = qq * 1024 + j2 * 512
                                    nc.tensor.matmul(
                                        qA[qq][:, j2 * 512:(j2 + 1) * 512],
                                        st, w2k[:, k, lo:lo + 512],
                                        start=False, stop=last)
                    signed_eng = os.environ.get("KERNEL_SIGNED", "act")
                    if "act" not in skip:
                        w1b = mid.tile([128, F1], BF_DT, tag="w1b", bufs=2)
                        nc.scalar.copy(out=w1b, in_=ps1[:, 0:F1])
                        for qq in range(2):
                            sl = slice(qq * 1024, (qq + 1) * 1024)
                            if signed_eng == "gpsimd":
                                nc.gpsimd.tensor_copy(w2b[:, sl], qA[qq])
                            else:
                                nc.scalar.copy(out=w2b[:, sl], in_=qA[qq])
                            nc.scalar.activation(
                                out=aw2[:, sl], in_=qA[qq],
                                func=mybir.ActivationFunctionType.Abs)

                    # ---- phase B: groups {q2, q3}, bias first
                    qB = [qp.tile([128, 1024], F32, tag="q", name=f"q{qq+2}")
                          for qq in range(2)]
                    if "gemm" not in skip:
                        for qq in range(2):
                            for j2 in range(2):
                                lo = 2048 + qq * 1024 + j2 * 512
                                nc.tensor.matmul(
                                    qB[qq][:, j2 * 512:(j2 + 1) * 512],
                                    ones[0:1, :], b2s[0:1, lo:lo + 512],
                                    start=True, stop=False)
                        for k in taps:
                            kh, kw = divmod(k, 3)
                            st = xmr[:, kh, kw:kw + 128]
                            last = k == taps[-1]
                            for qq in range(2):
                                for j2 in range(2):
                                    lo = 2048 + qq * 1024 + j2 * 512
                                    nc.tensor.matmul(
                                        qB[qq][:, j2 * 512:(j2 + 1) * 512],
                                        st, w2k[:, k, lo:lo + 512],
                                        start=False, stop=last)
                    if "act" not in skip:
                        for qq in range(2):
                            sl = slice(2048 + qq * 1024, 2048 + (qq + 1) * 1024)
                            if signed_eng == "gpsimd":
                                nc.gpsimd.tensor_copy(w2b[:, sl], qB[qq])
                            else:
                                nc.scalar.copy(out=w2b[:, sl], in_=qB[qq])
                            nc.scalar.activation(
                                out=aw2[:, sl], in_=qB[qq],
                                func=mybir.ActivationFunctionType.Abs)

                    if "apply" in skip or "act" in skip:
                        continue

                    with nc.allow_low_precision("bf16 intermediates validated "
                                                "against 2e-2 tolerance"):
                        # ---- t1: pat * w1b on Pool, path-major tile
                        t1 = mid.tile([128, 3, F1], BF_DT, tag="t1", bufs=2)
                        nc.gpsimd.tensor_mul(t1[:, 0, :], ypt, w1b)
                        nc.gpsimd.tensor_mul(t1[:, 1, :], mpt, w1b)
                        nc.gpsimd.tensor_mul(t1[:, 2, :], spt, w1b)
                        # ---- tap reduces on DVE (abs for m/s)
                        red = sml.tile([128, 3, C], BF_DT, tag="red")
                        nc.vector.tensor_reduce(
                            out=red[:, 0:1, :],
                            in_=t1[:, 0:1, :].rearrange(
                                "p a (c k) -> p a c k", k=K2),
                            axis=mybir.AxisListType.X, op=mybir.AluOpType.add)
                        nc.vector.tensor_reduce(
                            out=red[:, 1:3, :],
                            in_=t1[:, 1:3, :].rearrange(
                                "p a (c k) -> p a c k", k=K2),
                            axis=mybir.AxisListType.X, op=mybir.AluOpType.add,
                            apply_absolute_value=True)

                        # ---- materialize yr/mr/sr replicas into T
                        # (engine copies: a broadcast DMA shatters into 128B
                        # descriptors -- 1.6M per exec -- and its latency
                        # backpressures PE through the mid-pool recycle)
                        T = mid.tile([128, 3, C, C], BF_DT, tag="T")
                        rep_engs = {"mix": ("gpsimd", "act", "act"),
                                    "gpsimd": ("gpsimd",) * 3,
                                    "act": ("act",) * 3,
                                    "dma": ("dma",) * 3,
                                    "dve": ("dve",) * 3}[rep_mode]
                        for ci in range(3):
                            src = red[:, ci:ci + 1, :].to_broadcast([128, C, C])
                            dst = T[:, ci, :, :]
                            if rep_engs[ci] == "dma":
                                nc.sync.dma_start(out=dst, in_=src)
                            elif rep_engs[ci] == "gpsimd":
                                nc.gpsimd.tensor_copy(dst, src)
                            elif rep_engs[ci] == "act":
                                nc.scalar.copy(out=dst, in_=src)
                            # "dve": no materialize; mults below use bcast

                        # ---- t2 mults (2x-mode stride-1, in place on T)
                        w2v = w2b[:, :].rearrange("p (o c) -> p o c", c=C)
                        av2 = aw2[:, :].rearrange("p (o c) -> p o c", c=C)
                        if rep_mode == "dve":
                            for ci, wv in ((0, w2v), (1, av2), (2, av2)):
                                nc.vector.tensor_mul(
                                    T[:, ci, :, :], wv,
                                    red[:, ci:ci + 1, :].to_broadcast(
                                        [128, C, C]))
                        else:
                            for ci, wv in ((0, w2v), (1, av2), (2, av2)):
                                nc.vector.tensor_mul(
                                    T[:, ci, :, :], T[:, ci, :, :], wv)

                        # ---- batched tree-adds (all 3 paths per inst), then
                        # one fp32 tensor_reduce over the last 8
                        w = C
                        for _ in range(3):
                            w //= 2
                            nc.vector.tensor_add(
                                T[:, :, :, 0:w], T[:, :, :, 0:w],
                                T[:, :, :, w:2 * w])
                        acc3 = sml.tile([128, 3, C], F32, tag="acc3")
                        nc.vector.tensor_reduce(
                            out=acc3, in_=T[:, :, :, 0:w],
                            axis=mybir.AxisListType.X, op=mybir.AluOpType.add)

                        srec = sml.tile([128, C], F32, tag="srec")
                        nc.vector.reciprocal(out=srec, in_=acc3[:, 2, :])
                        my_t = sml.tile([128, C], F32, tag="my")
                        nc.vector.tensor_mul(my_t, acc3[:, 1, :], srec)
                        nc.sync.dma_start(out=y_d[h, :, :], in_=acc3[:, 0, :])
                        nc.sync.dma_start(out=my_d[h, :, :], in_=my_t)
    _split_multi_sync(nc)
    _dedup_ldweights(nc)
    return nc


def _row_gather(Wm, k):
    # rows of W (1152) feeding tap k for channels [x 0..63, m 0..63]
    idx = np.concatenate([np.arange(64) * 9 + k, 576 + np.arange(64) * 9 + k])
    return Wm[idx]


def _unfold(t):
    # t [B,C,H,W] fp32 -> patches [B,H,W,C*9] bf16 (index c*9 + kh*3 + kw)
    tp = np.pad(t, ((0, 0), (0, 0), (1, 1), (1, 1)), mode='edge').astype(BF)
    win = np.lib.stride_tricks.sliding_window_view(tp, (3, 3), axis=(2, 3))
    # win: [B,C,H,W,3,3] -> [B,H,W,C,3,3]
    return np.ascontiguousarray(win.transpose(0, 2, 3, 1, 4, 5)).reshape(B, H, W, C * 9)


def prepare_in_maps(x, m, s, W1, b1, W2, b2):
    x = np.asarray(x, np.float32); m = np.asarray(m, np.float32)
    s = np.asarray(s, np.float32)
    W1 = np.asarray(W1, np.float32); W2 = np.asarray(W2, np.float32)
    b1 = np.asarray(b1, np.float32); b2 = np.asarray(b2, np.float32)

    # W2 cols permuted from [c,o] to [o,c]; biases likewise
    W2p = W2.reshape(1152, C, C).transpose(0, 2, 1).reshape(1152, F2)
    b2p = b2.reshape(C, C).T.reshape(1, F2).astype(BF)
    w1k = np.stack([_row_gather(W1.astype(BF), k) for k in range(K2)])
    w2k = np.stack([_row_gather(W2p.astype(BF), k) for k in range(K2)])
    b1h = b1.reshape(1, F1).astype(BF)

    xmp = np.pad(np.concatenate([x, m], axis=1),
                 ((0, 0), (0, 0), (1, 1), (1, 1)), mode='edge').astype(BF)
    ypat = _unfold(x); mpat = _unfold(m); spat = _unfold(s)

    in_maps = []
    shards = []
    for core in range(N_CORES):
        b, half = divmod(core, 2)
        h0 = half * (H // 2)
        shards.append((b, h0))
        in_maps.append({
            "xm": np.ascontiguousarray(xmp[b, :, h0:h0 + ROWS + 2, :]),
            "ypat": np.ascontiguousarray(ypat[b, h0:h0 + ROWS].reshape(ROWS, W, F1)),
            "mpat": np.ascontiguousarray(mpat[b, h0:h0 + ROWS].reshape(ROWS, W, F1)),
            "spat": np.ascontiguousarray(spat[b, h0:h0 + ROWS].reshape(ROWS, W, F1)),
            "w1k": w1k, "w2k": w2k, "b1": b1h, "b2": b2p,
        })
    return in_maps, shards


_DEV_IN_CACHE = {}


def _time_hw(in_maps, reps, k1, k2, skip=()):
    """Steady-state per-execution time: build a reps-unrolled NEFF, stage
    inputs on device once, then measure the wall-clock slope between k1 and
    k2 pipelined dispatches. Returns ns per single kernel execution."""
    import time
    import jax
    from jax.sharding import Mesh, PartitionSpec, NamedSharding
    from jax.experimental.shard_map import shard_map
    from concourse.bass2jax import (_bass_exec_p, partition_id_tensor,
                                    install_neuronx_cc_hook)

    nc = build_program(reps=reps, skip=skip)
    install_neuronx_cc_hook()
    partition_name = (nc.partition_id_tensor.name
                      if nc.partition_id_tensor else None)
    in_names, out_names, out_avals, zero_shapes, zero_dtypes = [], [], [], [], []
    for alloc in nc.m.functions[0].allocations:
        if not isinstance(alloc, mybir.MemoryLocationSet):
            continue
        name = alloc.memorylocations[0].name
        if alloc.kind == "ExternalInput":
            if name != partition_name:
                in_names.append(name)
        elif alloc.kind == "ExternalOutput":
            out_names.append(name)
            shape = tuple(alloc.tensor_shape)
            dtype = mybir.dt.np(alloc.dtype)
            out_avals.append(jax.core.ShapedArray(shape, dtype))
            zero_shapes.append((N_CORES * shape[0], *shape[1:]))
            zero_dtypes.append(dtype)
    n_params = len(in_names)
    all_in_names = list(in_names) + list(out_names)
    if partition_name is not None:
        all_in_names.append(partition_name)

    def _body(*args):
        operands = list(args)
        if partition_name is not None:
            operands.append(partition_id_tensor())
        return tuple(_bass_exec_p.bind(
            *operands, out_avals=tuple(out_avals), in_names=tuple(all_in_names),
            out_names=tuple(out_names), lowering_input_output_aliases=(),
            sim_require_finite=True, sim_require_nnan=True, nc=nc))

    import functools
    import jax.numpy as jnp
    devices = jax.devices()[:N_CORES]
    mesh = Mesh(np.asarray(devices), ("core",))
    # no donation in the timing path: without declared IO aliasing the
    # custom-call results are fresh (uninit) buffers the NEFF fully writes,
    # so one static zero-set can serve every dispatch.
    fn = jax.jit(
        shard_map(_body, mesh=mesh,
                  in_specs=(PartitionSpec("core"),) * (n_params + len(out_names)),
                  out_specs=(PartitionSpec("core"),) * len(out_names),
                  check_rep=False),
        keep_unused=True)
    shard = NamedSharding(mesh, PartitionSpec("core"))
    global _DEV_IN_CACHE
    key = tuple(in_names)
    if _DEV_IN_CACHE.get("key") != key:
        _DEV_IN_CACHE = {"key": key, "arrs": [jax.device_put(
            np.concatenate([np.asarray(in_maps[c][nm]) for c in range(N_CORES)],
                           axis=0), shard) for nm in in_names]}
        jax.block_until_ready(_DEV_IN_CACHE["arrs"])
    dev_in = _DEV_IN_CACHE["arrs"]

    # allocate output buffers directly on device (no 32MB H2D per set)
    zfns = [jax.jit(functools.partial(jnp.zeros, s, d), out_shardings=shard)
            for s, d in zip(zero_shapes, zero_dtypes)]

    def fresh_zeros():
        zs = [f() for f in zfns]
        jax.block_until_ready(zs)
        return zs

    # warmup: trace + NEFF compile + first exec
    zs = fresh_zeros()
    outs = fn(*dev_in, *zs)
    jax.block_until_ready(outs)

    def timed(K):
        t0 = time.perf_counter()
        outs_l = [fn(*dev_in, *zs) for _ in range(K)]
        jax.block_until_ready(outs_l)
        return time.perf_counter() - t0

    slopes = []
    for _ in range(3):
        ta, tb = timed(k1), timed(k2)
        slopes.append((tb - ta) / ((k2 - k1) * reps))
    slopes.sort()
    per_exec = slopes[1]
    print(f"[time_hw] reps={reps} slopes(us)="
          f"{[int(s * 1e6) for s in slopes]} -> per-exec {per_exec*1e6:.0f}us")
    return int(per_exec * 1e9)


def kernel(x, m, s, W1, b1, W2, b2):
    in_maps, shards = prepare_in_maps(x, m, s, W1, b1, W2, b2)
    nc = build_program()
    res = run_bass_kernel_spmd(nc, in_maps, core_ids=list(range(N_CORES)),
                               trace=False)
    if os.environ.get("KERNEL_TIME"):
        reps = int(os.environ.get("KERNEL_TIME_REPS", "16"))
        ns = _time_hw(in_maps, reps=reps, k1=2, k2=8)
        with open("/tmp/kernel_exec_time.txt", "w") as f:
            f.write(str(ns))

    y = np.zeros((B, C, H, W), np.float32)
    m_y = np.zeros((B, C, H, W), np.float32)
    for core, (b, h0) in enumerate(shards):
        out = res.results[core]
        y[b, :, h0:h0 + ROWS, :] = np.asarray(
            out["y"], np.float32).transpose(2, 0, 1)
        m_y[b, :, h0:h0 + ROWS, :] = out["my"].transpose(2, 0, 1)
    return y, m_y, np.ones_like(m_y)


# revision 18
# speedup vs baseline: 1.7180x; 1.4769x over previous
"""Trainium2 Bass kernel for nn_Conv_M_49409303773352.

Strategy (data-parallel over batch x H-halves -> 8 shards):
  Per position p=(b,h,w): feat = [x-patches(576), m-patches(576)] (3x3, edge pad)
  w1 = feat@W1+b1 [576]; w2 = feat@W2+b2 [4096]
  yr_c = sum_k yp*w1 ; mr_c = sum_k |mp*w1| ; sr_c = sum_k |sp*w1|   (mp,sp>=0)
  y_o  = sum_c yr_c*w2[c,o] ; m_y = (sum_c mr_c|w2|)/(sum_c sr_c|w2|)

Device schedule (per 128-position row tile), v2 -- PE is the sole
bottleneck (~21.5us/row); everything else hides under it:
  PE   : 9-tap channel contraction (128 partitions = 64 x + 64 m chans)
         into PSUM, 1024-col chunks, 2 phases {w1,q0,q1} / {q2,q3};
         bias rows via ones-outer-product matmuls FIRST in each
         accumulation group (start=True), taps accumulate on top.
  ACT  : PSUM->SBUF bf16 downconverts: signed w2b (y path) AND |w2b|
         (shared by m,s paths via func=Abs).
  Pool : t1 mults (pat * w1b) into a path-major [128,3,576] tile.
  DVE  : t1 reduces (abs for m/s), three 2x-mode stride-1 t2 mults
         (in-place on materialized replicas), batched bf16 tree-adds
         (all 3 paths in one instruction) + final fp32 tensor_reduce,
         reciprocal + m_y multiply.
  DMA  : broadcast-materializes yr/mr/sr into the [128,3,64,64] T tile
         (replica per output channel) so the t2 mults avoid the 1x-mode
         broadcast-AP penalty on DVE.
All wire data bf16, fp32 accumulation in PSUM/final reduce.

build_program(reps=N) unrolls the whole row loop N times inside one NEFF
(outputs are recomputed identically each rep) so steady-state per-execution
time can be measured from dispatch-count slopes without per-dispatch RPC
overhead dominating.
"""
import sys
sys.path.insert(0, '/opt/trn_rl_repo')
import os
import numpy as np
import ml_dtypes

import concourse.bass as bass
import concourse.mybir as mybir
import concourse.tile as tile
from concourse.tile import TileContext
from concourse.vector_clock import ScopedClock
from concourse.bass_utils import run_bass_kernel_spmd

BF = ml_dtypes.bfloat16
BF_DT = mybir.dt.bfloat16
F32 = mybir.dt.float32

B, C, H, W = 4, 64, 128, 128
ROWS = int(os.environ.get("KERNEL_ROWS", "64"))   # output rows per core
N_CORES = 8
K2 = 9
F1 = 576          # K2*C
F2 = 4096         # C*C


# ---- walrus only accepts ONE sem wait per instruction: split the final drain
def _split_drain_and_barrier(self, tick_clock, wait_clock):
    nc = self.nc
    probe = nc.sync.nop()
    wait_clock.add_sem_waits(probe.ins, ScopedClock({None: tick_clock.global_clock}))
    waits = list(probe.ins.sync_info.on_wait)
    if len(waits) > 1:
        probe.ins.sync_info.on_wait = waits[:1]
        for w in waits[1:]:
            extra = nc.sync.nop()
            extra.ins.sync_info = probe.ins.sync_info.__class__(
                on_wait=[w], on_update=[])
    nc.sync.drain()
    nc.all_engine_barrier()
    assert self.sems is not None
    popped = nc._tile_sem_poison_stack.pop()
    assert popped is self._sem_poison
    nc.clear_and_free_semaphores(list(self.sems.allocated().values()))
    nc.all_engine_barrier()


tile.TileContext._drain_and_barrier = _split_drain_and_barrier


def _split_multi_sync(nc):
    """Walrus accepts one sync wait (and update) per instruction: hoist extras
    onto same-engine nops inserted just before (waits) / after (updates)."""
    def make_nop(engine, si_cls, waits=(), updates=()):
        bi = nc.engines[engine].nop()
        blk = nc.cur_bb.bb
        assert blk.instructions[-1] is bi.ins
        blk.instructions.pop()
        bi.ins.sync_info = si_cls(on_wait=list(waits), on_update=list(updates))
        return bi.ins

    for blk in nc.m.functions[0].blocks:
        out = []
        for inst in blk.instructions:
            si = getattr(inst, "sync_info", None)
            if si is None:
                out.append(inst)
                continue
            waits = list(si.on_wait or [])
            updates = list(si.on_update or [])
            extra_w = waits[:-1] if len(waits) > 1 else []
            extra_u = updates[1:] if len(updates) > 1 else []
            if extra_w:
                for w in extra_w:
                    out.append(make_nop(inst.engine, si.__class__, waits=[w]))
                si.on_wait = waits[-1:]
            out.append(inst)
            if extra_u:
                assert inst.opcode not in ("DMACopy", "DMATranspose"), \
                    "cannot defer DMA completion updates"
                si.on_update = updates[:1]
                for u in extra_u:
                    out.append(make_nop(inst.engine, si.__class__, updates=[u]))
        blk.instructions[:] = out


def _dedup_ldweights(nc):
    """Drop Ldweights whose stationary AP equals the previous Ldweights in
    the stream (PE array retains weights across the paired Matmults).
    Conservative: keep any Ldweights carrying sync waits/updates."""
    n_drop = 0
    for blk in nc.m.functions[0].blocks:
        out = []
        last_key = None
        for inst in blk.instructions:
            if inst.opcode == "Ldweights":
                si = getattr(inst, "sync_info", None)
                has_sync = si is not None and (si.on_wait or si.on_update)
                key = str(inst.ins[0])
                if key == last_key and not has_sync:
                    n_drop += 1
                    continue
                last_key = key
            out.append(inst)
        blk.instructions[:] = out
    return n_drop


def build_program(reps=1, skip=()):
    nc = bass.Bass()
    xm_d = nc.dram_tensor("xm", [2 * C, ROWS + 2, W + 2], BF_DT, kind="ExternalInput")
    ypat_d = nc.dram_tensor("ypat", [ROWS, W, F1], BF_DT, kind="ExternalInput")
    mpat_d = nc.dram_tensor("mpat", [ROWS, W, F1], BF_DT, kind="ExternalInput")
    spat_d = nc.dram_tensor("spat", [ROWS, W, F1], BF_DT, kind="ExternalInput")
    w1k_d = nc.dram_tensor("w1k", [K2, 128, F1], BF_DT, kind="ExternalInput")
    w2k_d = nc.dram_tensor("w2k", [K2, 128, F2], BF_DT, kind="ExternalInput")
    b1_d = nc.dram_tensor("b1", [1, F1], BF_DT, kind="ExternalInput")
    b2_d = nc.dram_tensor("b2", [1, F2], BF_DT, kind="ExternalInput")
    y_d = nc.dram_tensor("y", [ROWS, W, C], F32, kind="ExternalOutput")
    my_d = nc.dram_tensor("my", [ROWS, W, C], F32, kind="ExternalOutput")

    rep_mode = os.environ.get("KERNEL_REP", "dma")  # mix|dma|gpsimd|act|dve

    with TileContext(nc) as tc:
        with (
            tc.tile_pool(name="wts", bufs=1) as wts,
            tc.tile_pool(name="rows", bufs=4) as rows,
            tc.tile_pool(name="pats", bufs=3) as pats,
            tc.tile_pool(name="mid", bufs=2) as mid,
            tc.tile_pool(name="sml", bufs=3) as sml,
            tc.tile_pool(name="ps1p", bufs=1, space="PSUM") as ps1p,
            tc.tile_pool(name="qp", bufs=3, space="PSUM") as qp,
        ):
            w1k = wts.tile([128, K2, F1], BF_DT)
            nc.sync.dma_start(out=w1k, in_=w1k_d[:, :, :].rearrange("k p f -> p k f"))
            w2k = wts.tile([128, K2, F2], BF_DT)
            nc.sync.dma_start(out=w2k, in_=w2k_d[:, :, :].rearrange("k p f -> p k f"))
            b1s = wts.tile([1, F1], BF_DT)
            nc.sync.dma_start(out=b1s, in_=b1_d[:, :])
            b2s = wts.tile([1, F2], BF_DT)
            nc.sync.dma_start(out=b2s, in_=b2_d[:, :])
            ones = wts.tile([1, 128], BF_DT)
            nc.vector.memset(ones, 1.0)

            for rep in range(reps):
                for h in range(ROWS):
                    xmr = rows.tile([128, 3, W + 2], BF_DT)
                    nc.sync.dma_start(out=xmr, in_=xm_d[:, h:h + 3, :])
                    ypt = pats.tile([128, F1], BF_DT)
                    nc.sync.dma_start(out=ypt, in_=ypat_d[h, :, :])
                    mpt = pats.tile([128, F1], BF_DT)
                    nc.sync.dma_start(out=mpt, in_=mpat_d[h, :, :])
                    spt = pats.tile([128, F1], BF_DT)
                    nc.sync.dma_start(out=spt, in_=spat_d[h, :, :])

                    taps = tuple(range(K2))
                    w2b = mid.tile([128, F2], BF_DT, tag="w2b", bufs=2)
                    aw2 = mid.tile([128, F2], BF_DT, tag="aw2", bufs=2)

                    # ---- phase A: accumulation groups {w1(576), q0, q1};
                    # bias rows first (start=True) via ones outer-product,
                    # then the 9 taps accumulate on top.
                    ps1 = ps1p.tile([128, 1024], F32, tag="ps1")
                    qA = [qp.tile([128, 1024], F32, tag="q", name=f"q{qq}")
                          for qq in range(2)]
                    if "gemm" not in skip:
                        for lo, hi in ((0, 512), (512, F1)):
                            nc.tensor.matmul(ps1[:, lo:hi], ones[0:1, :],
                                             b1s[0:1, lo:hi], start=True,
                                             stop=False)
                        for qq in range(2):
                            for j2 in range(2):
                                lo = qq * 1024 + j2 * 512
                                nc.tensor.matmul(
                                    qA[qq][:, j2 * 512:(j2 + 1) * 512],
                                    ones[0:1, :], b2s[0:1, lo:lo + 512],
                                    start=True, stop=False)
                        for k in taps:
                            kh, kw = divmod(k, 3)
                            st = xmr[:, kh, kw:kw + 128]
                            last = k == taps[-1]
                            for lo, hi in ((0, 512), (512, F1)):
                                nc.tensor.matmul(ps1[:, lo:hi], st,
                                                 w1k[:, k, lo:hi],
                                                 start=False, stop=last)
                            for qq in range(2):
                                for j2 in range(2):
                                    lo = qq * 1024 + j2 * 512
                                    nc.tensor.matmul(
                                        qA[qq][:, j2 * 512:(j2 + 1) * 512],
                                        st, w2k[:, k, lo:lo + 512],
                                        start=False, stop=last)
                    signed_eng = os.environ.get("KERNEL_SIGNED", "act")
                    if "act" not in skip:
                        w1b = mid.tile([128, F1], BF_DT, tag="w1b", bufs=2)
                        nc.scalar.copy(out=w1b, in_=ps1[:, 0:F1])
                        for qq in range(2):
                            sl = slice(qq * 1024, (qq + 1) * 1024)
                            if signed_eng == "gpsimd":
                                nc.gpsimd.tensor_copy(w2b[:, sl], qA[qq])
                            else:
                                nc.scalar.copy(out=w2b[:, sl], in_=qA[qq])
                            nc.scalar.activation(
                                out=aw2[:, sl], in_=qA[qq],
                                func=mybir.ActivationFunctionType.Abs)

                    # ---- phase B: groups {q2, q3}, bias first
                    qB = [qp.tile([128, 1024], F32, tag="q", name=f"q{qq+2}")
                          for qq in range(2)]
                    if "gemm" not in skip:
                        for qq in range(2):
                            for j2 in range(2):
                                lo = 2048 + qq * 1024 + j2 * 512
                                nc.tensor.matmul(
                                    qB[qq][:, j2 * 512:(j2 + 1) * 512],
                                    ones[0:1, :], b2s[0:1, lo:lo + 512],
                                    start=True, stop=False)
                        for k in taps:
                            kh, kw = divmod(k, 3)
                            st = xmr[:, kh, kw:kw + 128]
                            last = k == taps[-1]
                            for qq in range(2):
                                for j2 in range(2):
                                    lo = 2048 + qq * 1024 + j2 * 512
                                    nc.tensor.matmul(
                                        qB[qq][:, j2 * 512:(j2 + 1) * 512],
                                        st, w2k[:, k, lo:lo + 512],
                                        start=False, stop=last)
                    if "act" not in skip:
                        for qq in range(2):
                            sl = slice(2048 + qq * 1024, 2048 + (qq + 1) * 1024)
                            if signed_eng == "gpsimd":
                                nc.gpsimd.tensor_copy(w2b[:, sl], qB[qq])
                            else:
                                nc.scalar.copy(out=w2b[:, sl], in_=qB[qq])
                            nc.scalar.activation(
                                out=aw2[:, sl], in_=qB[qq],
                                func=mybir.ActivationFunctionType.Abs)

                    if "apply" in skip or "act" in skip:
                        continue

                    with nc.allow_low_precision("bf16 intermediates validated "
                                                "against 2e-2 tolerance"):
                        # ---- t1: pat * w1b on Pool, path-major tile
                        t1 = mid.tile([128, 3, F1], BF_DT, tag="t1", bufs=2)
                        nc.gpsimd.tensor_mul(t1[:, 0, :], ypt, w1b)
                        nc.gpsimd.tensor_mul(t1[:, 1, :], mpt, w1b)
                        nc.gpsimd.tensor_mul(t1[:, 2, :], spt, w1b)
                        # ---- tap reduces on DVE (abs for m/s)
                        red = sml.tile([128, 3, C], BF_DT, tag="red")
                        nc.vector.tensor_reduce(
                            out=red[:, 0:1, :],
                            in_=t1[:, 0:1, :].rearrange(
                                "p a (c k) -> p a c k", k=K2),
                            axis=mybir.AxisListType.X, op=mybir.AluOpType.add)
                        nc.vector.tensor_reduce(
                            out=red[:, 1:3, :],
                            in_=t1[:, 1:3, :].rearrange(
                                "p a (c k) -> p a c k", k=K2),
                            axis=mybir.AxisListType.X, op=mybir.AluOpType.add,
                            apply_absolute_value=True)

                        # ---- t2 mults. The replica (redb repeated along o)
                        # is materialized by ACT into T's o-half-1 region only
                        # (2048 cols per path; a full broadcast DMA shatters
                        # into 128B descriptors -- 1.6M per exec -- and its
                        # steady-state backlog costs ~200us/exec). Then per
                        # path: half0 = replica * w2[0:32] (reads half1,
                        # writes half0), half1 *= w2[32:64] in place. All
                        # stride-1 bf16 -> DVE 2x mode.
                        T = mid.tile([128, 3, C, C], BF_DT, tag="T")
                        w2v = w2b[:, :].rearrange("p (o c) -> p o c", c=C)
                        av2 = aw2[:, :].rearrange("p (o c) -> p o c", c=C)
                        Ch = C // 2
                        if rep_mode == "dve":
                            for ci, wv in ((0, w2v), (1, av2), (2, av2)):
                                nc.vector.tensor_mul(
                                    T[:, ci, :, :], wv,
                                    red[:, ci:ci + 1, :].to_broadcast(
                                        [128, C, C]))
                        else:
                            for ci in range(3):
                                src = red[:, ci:ci + 1, :].to_broadcast(
                                    [128, Ch, C])
                                dst = T[:, ci, Ch:C, :]
                                if rep_mode == "dma":
                                    nc.sync.dma_start(out=dst, in_=src)
                                else:
                                    nc.scalar.copy(out=dst, in_=src)
                            for ci, wv in ((0, w2v), (1, av2), (2, av2)):
                                nc.vector.tensor_mul(
                                    T[:, ci, 0:Ch, :], T[:, ci, Ch:C, :],
                                    wv[:, 0:Ch, :])
                                nc.vector.tensor_mul(
                                    T[:, ci, Ch:C, :], T[:, ci, Ch:C, :],
                                    wv[:, Ch:C, :])

                        # ---- batched tree-adds (all 3 paths per inst), then
                        # one fp32 tensor_reduce over the last 8
                        w = C
                        for _ in range(3):
                            w //= 2
                            nc.vector.tensor_add(
                                T[:, :, :, 0:w], T[:, :, :, 0:w],
                                T[:, :, :, w:2 * w])
                        acc3 = sml.tile([128, 3, C], F32, tag="acc3")
                        nc.vector.tensor_reduce(
                            out=acc3, in_=T[:, :, :, 0:w],
                            axis=mybir.AxisListType.X, op=mybir.AluOpType.add)

                        srec = sml.tile([128, C], F32, tag="srec")
                        nc.vector.reciprocal(out=srec, in_=acc3[:, 2, :])
                        my_t = sml.tile([128, C], F32, tag="my")
                        nc.vector.tensor_mul(my_t, acc3[:, 1, :], srec)
                        nc.sync.dma_start(out=y_d[h, :, :], in_=acc3[:, 0, :])
                        nc.sync.dma_start(out=my_d[h, :, :], in_=my_t)
    _split_multi_sync(nc)
    _dedup_ldweights(nc)
    return nc


def _row_gather(Wm, k):
    # rows of W (1152) feeding tap k for channels [x 0..63, m 0..63]
    idx = np.concatenate([np.arange(64) * 9 + k, 576 + np.arange(64) * 9 + k])
    return Wm[idx]


def _unfold(t):
    # t [B,C,H,W] fp32 -> patches [B,H,W,C*9] bf16 (index c*9 + kh*3 + kw)
    tp = np.pad(t, ((0, 0), (0, 0), (1, 1), (1, 1)), mode='edge').astype(BF)
    win = np.lib.stride_tricks.sliding_window_view(tp, (3, 3), axis=(2, 3))
    # win: [B,C,H,W,3,3] -> [B,H,W,C,3,3]
    return np.ascontiguousarray(win.transpose(0, 2, 3, 1, 4, 5)).reshape(B, H, W, C * 9)


def prepare_in_maps(x, m, s, W1, b1, W2, b2):
    x = np.asarray(x, np.float32); m = np.asarray(m, np.float32)
    s = np.asarray(s, np.float32)
    W1 = np.asarray(W1, np.float32); W2 = np.asarray(W2, np.float32)
    b1 = np.asarray(b1, np.float32); b2 = np.asarray(b2, np.float32)

    # W2 cols permuted from [c,o] to [o,c]; biases likewise
    W2p = W2.reshape(1152, C, C).transpose(0, 2, 1).reshape(1152, F2)
    b2p = b2.reshape(C, C).T.reshape(1, F2).astype(BF)
    w1k = np.stack([_row_gather(W1.astype(BF), k) for k in range(K2)])
    w2k = np.stack([_row_gather(W2p.astype(BF), k) for k in range(K2)])
    b1h = b1.reshape(1, F1).astype(BF)

    xmp = np.pad(np.concatenate([x, m], axis=1),
                 ((0, 0), (0, 0), (1, 1), (1, 1)), mode='edge').astype(BF)
    ypat = _unfold(x); mpat = _unfold(m); spat = _unfold(s)

    in_maps = []
    shards = []
    for core in range(N_CORES):
        b, half = divmod(core, 2)
        h0 = half * (H // 2)
        shards.append((b, h0))
        in_maps.append({
            "xm": np.ascontiguousarray(xmp[b, :, h0:h0 + ROWS + 2, :]),
            "ypat": np.ascontiguousarray(ypat[b, h0:h0 + ROWS].reshape(ROWS, W, F1)),
            "mpat": np.ascontiguousarray(mpat[b, h0:h0 + ROWS].reshape(ROWS, W, F1)),
            "spat": np.ascontiguousarray(spat[b, h0:h0 + ROWS].reshape(ROWS, W, F1)),
            "w1k": w1k, "w2k": w2k, "b1": b1h, "b2": b2p,
        })
    return in_maps, shards


_DEV_IN_CACHE = {}


def _time_hw(in_maps, reps, k1, k2, skip=()):
    """Steady-state per-execution time: build a reps-unrolled NEFF, stage
    inputs on device once, then measure the wall-clock slope between k1 and
    k2 pipelined dispatches. Returns ns per single kernel execution."""
    import time
    import jax
    from jax.sharding import Mesh, PartitionSpec, NamedSharding
    from jax.experimental.shard_map import shard_map
    from concourse.bass2jax import (_bass_exec_p, partition_id_tensor,
                                    install_neuronx_cc_hook)

    nc = build_program(reps=reps, skip=skip)
    install_neuronx_cc_hook()
    partition_name = (nc.partition_id_tensor.name
                      if nc.partition_id_tensor else None)
    in_names, out_names, out_avals, zero_shapes, zero_dtypes = [], [], [], [], []
    for alloc in nc.m.functions[0].allocations:
        if not isinstance(alloc, mybir.MemoryLocationSet):
            continue
        name = alloc.memorylocations[0].name
        if alloc.kind == "ExternalInput":
            if name != partition_name:
                in_names.append(name)
        elif alloc.kind == "ExternalOutput":
            out_names.append(name)
            shape = tuple(alloc.tensor_shape)
            dtype = mybir.dt.np(alloc.dtype)
            out_avals.append(jax.core.ShapedArray(shape, dtype))
            zero_shapes.append((N_CORES * shape[0], *shape[1:]))
            zero_dtypes.append(dtype)
    n_params = len(in_names)
    all_in_names = list(in_names) + list(out_names)
    if partition_name is not None:
        all_in_names.append(partition_name)

    def _body(*args):
        operands = list(args)
        if partition_name is not None:
            operands.append(partition_id_tensor())
        return tuple(_bass_exec_p.bind(
            *operands, out_avals=tuple(out_avals), in_names=tuple(all_in_names),
            out_names=tuple(out_names), lowering_input_output_aliases=(),
            sim_require_finite=True, sim_require_nnan=True, nc=nc))

    import functools
    import jax.numpy as jnp
    devices = jax.devices()[:N_CORES]
    mesh = Mesh(np.asarray(devices), ("core",))
    # no donation in the timing path: without declared IO aliasing the
    # custom-call results are fresh (uninit) buffers the NEFF fully writes,
    # so one static zero-set can serve every dispatch.
    fn = jax.jit(
        shard_map(_body, mesh=mesh,
                  in_specs=(PartitionSpec("core"),) * (n_params + len(out_names)),
                  out_specs=(PartitionSpec("core"),) * len(out_names),
                  check_rep=False),
        keep_unused=True)
    shard = NamedSharding(mesh, PartitionSpec("core"))
    global _DEV_IN_CACHE
    key = tuple(in_names)
    if _DEV_IN_CACHE.get("key") != key:
        _DEV_IN_CACHE = {"key": key, "arrs": [jax.device_put(
            np.concatenate([np.asarray(in_maps[c][nm]) for c in range(N_CORES)],
                           axis=0), shard) for nm in in_names]}
        jax.block_until_ready(_DEV_IN_CACHE["arrs"])
    dev_in = _DEV_IN_CACHE["arrs"]

    # allocate output buffers directly on device (no 32MB H2D per set)
    zfns = [jax.jit(functools.partial(jnp.zeros, s, d), out_shardings=shard)
            for s, d in zip(zero_shapes, zero_dtypes)]

    def fresh_zeros():
        zs = [f() for f in zfns]
        jax.block_until_ready(zs)
        return zs

    # warmup: trace + NEFF compile + first exec
    zs = fresh_zeros()
    outs = fn(*dev_in, *zs)
    jax.block_until_ready(outs)

    def timed(K):
        t0 = time.perf_counter()
        outs_l = [fn(*dev_in, *zs) for _ in range(K)]
        jax.block_until_ready(outs_l)
        return time.perf_counter() - t0

    slopes = []
    for _ in range(3):
        ta, tb = timed(k1), timed(k2)
        slopes.append((tb - ta) / ((k2 - k1) * reps))
    slopes.sort()
    per_exec = slopes[1]
    print(f"[time_hw] reps={reps} slopes(us)="
          f"{[int(s * 1e6) for s in slopes]} -> per-exec {per_exec*1e6:.0f}us")
    return int(per_exec * 1e9)


def kernel(x, m, s, W1, b1, W2, b2):
    in_maps, shards = prepare_in_maps(x, m, s, W1, b1, W2, b2)
    nc = build_program()
    res = run_bass_kernel_spmd(nc, in_maps, core_ids=list(range(N_CORES)),
                               trace=False)
    if os.environ.get("KERNEL_TIME"):
        reps = int(os.environ.get("KERNEL_TIME_REPS", "16"))
        ns = _time_hw(in_maps, reps=reps, k1=2, k2=8)
        with open("/tmp/kernel_exec_time.txt", "w") as f:
            f.write(str(ns))

    y = np.zeros((B, C, H, W), np.float32)
    m_y = np.zeros((B, C, H, W), np.float32)
    for core, (b, h0) in enumerate(shards):
        out = res.results[core]
        y[b, :, h0:h0 + ROWS, :] = np.asarray(
            out["y"], np.float32).transpose(2, 0, 1)
        m_y[b, :, h0:h0 + ROWS, :] = out["my"].transpose(2, 0, 1)
    return y, m_y, np.ones_like(m_y)
